# revision 9
# baseline (speedup 1.0000x reference)
"""Bezier soft-disk renderer on 8 Trainium2 NeuronCores.

Strategy (data-parallel over paths + associative over-compositing):
  Each core gets 128 of the 1024 paths. Front-to-back compositing
    canvas <- canvas*(1-m_p) + c_p*m_p
  is an affine map per pixel, so a shard of 128 consecutive paths
  composes to  canvas_out = canvas_in * A_s + B_s  with
    A_s = prod_p (1-m_p)
    B_s = sum_p c_p m_p prod_{q>p} (1-m_q).
  On-device, per core (paths on the 128 SBUF partitions):
    d2   = (gx-cx)^2 + (gy-cy)^2            (DVE tensor_scalar + ACT Square)
    m0   = Sigmoid(-50*sqrt(d2) + 50*r)     (ACT, per-partition bias)
    lg   = Ln(1 - alpha*m0)                 (ACT, per-partition scale)
    SS_k = sum_{q>=k} lg_q                  (TensorE, triangular ones matmul)
    V    = Exp(SS)                          (ACT)  -> V_0 = A_s
    B_s  = D^T @ V + c_last @ ones          (TensorE; D_k = c_{k-1}-c_k, D_0=-c_0,
                                             4th output row = V_0 = A_s)
  Host work is only shard/gather + the 8-term affine combine.
"""

import sys

if "/opt/trn_rl_repo" not in sys.path:
    sys.path.insert(0, "/opt/trn_rl_repo")

import numpy as np
from contextlib import ExitStack

H = W = 224
NPX = H * W
N_PATHS = 1024
PPC = 128           # paths per core
NCORES = 8
NSEG = 4
NSAMP = 50
NT = NSAMP - 1      # 49 samples per segment
NPTS = NSEG * NT    # 196 samples per path
INV_SOFT = 50.0     # 1/SOFTNESS
CHUNK_ROWS = 56
CH_PX = CHUNK_ROWS * W      # 12544
NCHUNK = H // CHUNK_ROWS    # 4
BLK = 512                   # matmul moving-dim block (one PSUM bank)

_compiled = {}
last_results = None


def _build_nc():
    import concourse.tile as tile
    from concourse import bacc, mybir

    f32 = mybir.dt.float32
    f32r = mybir.dt.float32r
    ACT = mybir.ActivationFunctionType
    ALU = mybir.AluOpType

    nc = bacc.Bacc("TRN2", target_bir_lowering=False, debug=False,
                   num_devices=NCORES)

    cp_d = nc.dram_tensor("cp", [PPC, NSEG * 4 * 2], f32, kind="ExternalInput").ap()
    col_d = nc.dram_tensor("col", [PPC, 4], f32, kind="ExternalInput").ap()
    lin_d = nc.dram_tensor("lin_t", [PPC, W], f32, kind="ExternalInput").ap()
    bas_d = nc.dram_tensor("basis", [PPC, 4 * NT], f32, kind="ExternalInput").ap()
    tri_d = nc.dram_tensor("tri", [PPC, PPC], f32, kind="ExternalInput").ap()
    ab_d = nc.dram_tensor("AB", [4, NPX], f32, kind="ExternalOutput").ap()

    with ExitStack() as ctx:
        tc = ctx.enter_context(tile.TileContext(nc))

        singles = ctx.enter_context(tc.tile_pool(name="singles", bufs=1))
        setup = ctx.enter_context(tc.tile_pool(name="setup", bufs=1))
        work = ctx.enter_context(tc.tile_pool(name="work", bufs=2))
        lgpool = ctx.enter_context(tc.tile_pool(name="lg", bufs=1))
        vpool = ctx.enter_context(tc.tile_pool(name="vp", bufs=3))
        bstage = ctx.enter_context(tc.tile_pool(name="bst", bufs=3))
        ps_ss = ctx.enter_context(tc.tile_pool(name="pss", bufs=2, space="PSUM"))
        ps_b = ctx.enter_context(tc.tile_pool(name="psb", bufs=4, space="PSUM"))

        # ---- load inputs -------------------------------------------------
        cp_sb = singles.tile([PPC, NSEG * 4 * 2], f32)
        nc.sync.dma_start(cp_sb[:], cp_d)
        col_sb = singles.tile([PPC, 4], f32)
        nc.sync.dma_start(col_sb[:], col_d)
        lin_sb = singles.tile([PPC, W], f32)
        nc.sync.dma_start(lin_sb[:], lin_d)
        bas_sb = singles.tile([PPC, 4 * NT], f32)
        nc.sync.dma_start(bas_sb[:], bas_d)
        tri_sb = singles.tile([PPC, PPC], f32)
        nc.sync.dma_start(tri_sb[:], tri_d)

        # ---- bezier samples: pts[p,s,t,e] = sum_c basis[t,c]*cp[p,s,c,e] -
        cp4 = cp_sb[:].rearrange("p (s c e) -> p s c e", s=NSEG, c=4, e=2)
        bas4 = bas_sb[:].rearrange("p (c t) -> p c t", c=4, t=NT)
        prods = []
        for c in range(4):
            pc = setup.tile([PPC, NSEG, NT, 2], f32, tag=f"prod{c}")
            cpv = cp4[:, :, c, :].unsqueeze(2).broadcast_to([PPC, NSEG, NT, 2])
            bv = (bas4[:, c, :].unsqueeze(1).unsqueeze(3)
                  .broadcast_to([PPC, NSEG, NT, 2]))
            nc.vector.tensor_mul(pc[:], cpv, bv)
            prods.append(pc)
        s01 = setup.tile([PPC, NSEG, NT, 2], f32)
        nc.vector.tensor_add(s01[:], prods[0][:], prods[1][:])
        s23 = setup.tile([PPC, NSEG, NT, 2], f32)
        nc.vector.tensor_add(s23[:], prods[2][:], prods[3][:])
        pts = setup.tile([PPC, NSEG, NT, 2], f32)
        nc.vector.tensor_add(pts[:], s01[:], s23[:])

        ptx = pts[:, :, :, 0]   # [p, 4, 49]
        pty = pts[:, :, :, 1]

        # ---- centers (negated means) ------------------------------------
        sumx = setup.tile([PPC, 1], f32)
        nc.vector.tensor_reduce(sumx[:], ptx, axis=mybir.AxisListType.XY,
                                op=ALU.add)
        sumy = setup.tile([PPC, 1], f32)
        nc.vector.tensor_reduce(sumy[:], pty, axis=mybir.AxisListType.XY,
                                op=ALU.add)
        neg_cx = setup.tile([PPC, 1], f32)
        nc.vector.tensor_scalar_mul(neg_cx[:], sumx[:], -1.0 / NPTS)
        neg_cy = setup.tile([PPC, 1], f32)
        nc.vector.tensor_scalar_mul(neg_cy[:], sumy[:], -1.0 / NPTS)

        # ---- avg radius -> r50 = 50 * mean ||pts - c|| -------------------
        sqx = setup.tile([PPC, NSEG, NT], f32)
        nc.scalar.activation(sqx[:], ptx, ACT.Square, bias=neg_cx[:])
        sqy = setup.tile([PPC, NSEG, NT], f32)
        nc.scalar.activation(sqy[:], pty, ACT.Square, bias=neg_cy[:])
        d2p = setup.tile([PPC, NSEG, NT], f32)
        nc.vector.tensor_add(d2p[:], sqx[:], sqy[:])
        sp = setup.tile([PPC, NSEG, NT], f32)
        rsum = setup.tile([PPC, 1], f32)
        nc.scalar.activation(sp[:], d2p[:], ACT.Sqrt, accum_out=rsum[:])
        r50 = setup.tile([PPC, 1], f32)
        nc.vector.tensor_scalar_mul(r50[:], rsum[:], INV_SOFT / NPTS)

        # ---- per-path alpha and color-diff matmul weights ----------------
        neg_alpha = setup.tile([PPC, 1], f32)
        nc.vector.tensor_scalar_mul(neg_alpha[:], col_sb[:, 3:4], -1.0)

        csh = setup.tile([PPC, 3], f32)       # c_{k-1} (0 for k=0)
        nc.vector.memset(csh[0:1, :], 0.0)
        nc.sync.dma_start(csh[1:PPC, :], col_sb[0:PPC - 1, 0:3])
        d4f = setup.tile([PPC, 4], f32)       # cols 0-2: D, col 3: e_0 (A row)
        nc.vector.tensor_sub(d4f[:, 0:3], csh[:], col_sb[:, 0:3])
        nc.vector.memset(d4f[:, 3:4], 0.0)
        nc.vector.memset(d4f[0:1, 3:4], 1.0)
        d4 = setup.tile([PPC, 4], f32r)
        nc.vector.tensor_copy(d4[:], d4f[:])

        cl_f = setup.tile([1, 4], f32)
        nc.vector.memset(cl_f[:], 0.0)
        nc.sync.dma_start(cl_f[0:1, 0:3], col_sb[PPC - 1:PPC, 0:3])
        cl4 = setup.tile([1, 4], f32r)        # c_last row (0 in A column)
        nc.vector.tensor_copy(cl4[:], cl_f[:])
        ones_f = setup.tile([1, BLK], f32)
        nc.vector.memset(ones_f[:], 1.0)
        ones_row = setup.tile([1, BLK], f32r)
        nc.vector.tensor_copy(ones_row[:], ones_f[:])
        trir = singles.tile([PPC, PPC], f32r)
        nc.vector.tensor_copy(trir[:], tri_sb[:])

        # ---- separable squared distances --------------------------------
        dx2 = singles.tile([PPC, W], f32)
        nc.scalar.activation(dx2[:], lin_sb[:], ACT.Square, bias=neg_cx[:])
        dy2 = singles.tile([PPC, W], f32)
        nc.scalar.activation(dy2[:], lin_sb[:], ACT.Square, bias=neg_cy[:])

        # ---- main loop ---------------------------------------------------
        for ch in range(NCHUNK):
            t = work.tile([PPC, CH_PX], f32, tag="work")
            for r in range(CHUNK_ROWS):
                i = ch * CHUNK_ROWS + r
                nc.vector.tensor_scalar_add(
                    t[:, r * W:(r + 1) * W], dx2[:], dy2[:, i:i + 1])
            # d2 -> dist -> m0 in place, lg to its own (f32r) tile
            nc.scalar.activation(t[:], t[:], ACT.Sqrt)
            nc.scalar.activation(t[:], t[:], ACT.Sigmoid,
                                 bias=r50[:], scale=-INV_SOFT)
            lg = lgpool.tile([PPC, CH_PX], f32r, tag="lg")
            nc.scalar.activation(lg[:], t[:], ACT.Ln,
                                 bias=1.0, scale=neg_alpha[:])

            nblk = (CH_PX + BLK - 1) // BLK
            for b in range(nblk):
                lo = b * BLK
                bw = min(BLK, CH_PX - lo)
                px0 = ch * CH_PX + lo

                ss = ps_ss.tile([PPC, BLK], f32, tag="ss")
                nc.tensor.matmul(ss[:, :bw], trir[:],
                                 lg[:, lo:lo + bw],
                                 start=True, stop=True)
                v = vpool.tile([PPC, BLK], f32r, tag="v")
                nc.scalar.activation(v[:, :bw], ss[:, :bw], ACT.Exp)

                bp = ps_b.tile([4, BLK], f32, tag="bp")
                nc.tensor.matmul(bp[:, :bw], d4[:],
                                 v[:, :bw],
                                 start=True, stop=False)
                nc.tensor.matmul(bp[:, :bw], cl4[:],
                                 ones_row[:, :bw],
                                 start=False, stop=True)
                bs = bstage.tile([4, BLK], f32, tag="bs")
                nc.vector.tensor_copy(bs[:, :bw], bp[:, :bw])
                nc.sync.dma_start(ab_d[:, px0:px0 + bw], bs[:, :bw])

    nc.compile()
    return nc


def _get_nc():
    if "nc" not in _compiled:
        _compiled["nc"] = _build_nc()
    return _compiled["nc"]


def _bezier_basis():
    t = np.linspace(0.0, 1.0, NSAMP, dtype=np.float32)[:-1]
    mt = 1.0 - t
    return np.stack([mt ** 3, 3.0 * mt ** 2 * t, 3.0 * mt * t ** 2, t ** 3],
                    axis=-1).astype(np.float32)  # (49, 4)


def kernel(paths_control_points, colors):
    global last_results
    from concourse.bass_utils import run_bass_kernel_spmd

    cp = np.ascontiguousarray(paths_control_points, dtype=np.float32)
    col = np.ascontiguousarray(colors, dtype=np.float32)

    basis = _bezier_basis()                       # (49, 4)
    bas_in = np.broadcast_to(basis.T.reshape(1, 4 * NT),
                             (PPC, 4 * NT)).copy()  # rows: c-major
    lin = np.linspace(0.0, 1.0, W, dtype=np.float32)
    lin_in = np.broadcast_to(lin, (PPC, W)).copy()
    q = np.arange(PPC)
    tri = (q[:, None] >= q[None, :]).astype(np.float32)  # tri[q,k] = q>=k

    nc = _get_nc()
    in_maps = []
    for s in range(NCORES):
        sl = slice(s * PPC, (s + 1) * PPC)
        in_maps.append({
            "cp": cp[sl].reshape(PPC, NSEG * 4 * 2).copy(),
            "col": col[sl].copy(),
            "lin_t": lin_in,
            "basis": bas_in,
            "tri": tri,
        })

    res = run_bass_kernel_spmd(nc, in_maps, core_ids=list(range(NCORES)))
    last_results = res

    canvas = np.ones((3, H, W), dtype=np.float32)
    for s in range(NCORES):
        ab = res.results[s]["AB"]
        a = ab[3].reshape(H, W)
        b = ab[0:3].reshape(3, H, W)
        canvas = canvas * a[None] + b
    return canvas.astype(np.float32)


# revision 18
# speedup vs baseline: 1.2388x; 1.2388x over previous
"""Bezier soft-disk renderer on 8 Trainium2 NeuronCores.

Strategy (data-parallel over paths + associative over-compositing):
  Each core gets 128 of the 1024 paths. Front-to-back compositing
    canvas <- canvas*(1-m_p) + c_p*m_p
  is an affine map per pixel, so a shard of 128 consecutive paths
  composes to  canvas_out = canvas_in * A_s + B_s  with
    A_s = prod_p (1-m_p)
    B_s = sum_p c_p m_p prod_{q>p} (1-m_q).
  On-device, per core (paths on the 128 SBUF partitions):
    d2   = (gx-cx)^2 + (gy-cy)^2            (DVE tensor_scalar + ACT Square)
    m0   = Sigmoid(-50*sqrt(d2) + 50*r)     (ACT, per-partition bias)
    lg   = Ln(1 - alpha*m0)                 (ACT, per-partition scale)
    SS_k = sum_{q>=k} lg_q                  (TensorE, triangular ones matmul)
    V    = Exp(SS)                          (ACT)  -> V_0 = A_s
    B_s  = D^T @ V + c_last @ ones          (TensorE; D_k = c_{k-1}-c_k, D_0=-c_0,
                                             4th output row = V_0 = A_s)
  Host work is only shard/gather + the 8-term affine combine.
"""

import sys

if "/opt/trn_rl_repo" not in sys.path:
    sys.path.insert(0, "/opt/trn_rl_repo")

import numpy as np
from contextlib import ExitStack

H = W = 224
NPX = H * W
N_PATHS = 1024
PPC = 128           # paths per core
NCORES = 8
NSEG = 4
NSAMP = 50
NT = NSAMP - 1      # 49 samples per segment
NPTS = NSEG * NT    # 196 samples per path
INV_SOFT = 50.0     # 1/SOFTNESS
CHUNKS = [80, 72, 72]       # rows per chunk (sum = 224)
MAXCH_PX = max(CHUNKS) * W  # biggest chunk, sizes the work tiles
BLK = 512                   # matmul moving-dim block (one PSUM bank)
XBLK = 1024                 # exp/copy/DMA grouping (two PSUM banks)

_compiled = {}
last_results = None


def _build_nc():
    import concourse.tile as tile
    from concourse import bacc, mybir

    f32 = mybir.dt.float32
    f32r = mybir.dt.float32r
    bf16 = mybir.dt.bfloat16
    ACT = mybir.ActivationFunctionType
    ALU = mybir.AluOpType

    nc = bacc.Bacc("TRN2", target_bir_lowering=False, debug=False,
                   num_devices=NCORES)

    cp_d = nc.dram_tensor("cp", [PPC, NSEG * 4 * 2], f32, kind="ExternalInput").ap()
    col_d = nc.dram_tensor("col", [PPC, 4], f32, kind="ExternalInput").ap()
    lin_d = nc.dram_tensor("lin_t", [PPC, W], f32, kind="ExternalInput").ap()
    bas_d = nc.dram_tensor("basis", [PPC, 4 * NT], f32, kind="ExternalInput").ap()
    tri_d = nc.dram_tensor("tri", [PPC, PPC], bf16, kind="ExternalInput").ap()
    ab_d = nc.dram_tensor("AB", [4, NPX], f32, kind="ExternalOutput").ap()

    with ExitStack() as ctx:
        tc = ctx.enter_context(tile.TileContext(nc))

        singles = ctx.enter_context(tc.tile_pool(name="singles", bufs=1))
        setup = ctx.enter_context(tc.tile_pool(name="setup", bufs=1))
        work = ctx.enter_context(tc.tile_pool(name="work", bufs=1))
        lgpool = ctx.enter_context(tc.tile_pool(name="lg", bufs=1))
        vpool = ctx.enter_context(tc.tile_pool(name="vp", bufs=3))
        bstage = ctx.enter_context(tc.tile_pool(name="bst", bufs=3))
        ps_ss = ctx.enter_context(tc.tile_pool(name="pss", bufs=2, space="PSUM"))
        ps_b = ctx.enter_context(tc.tile_pool(name="psb", bufs=2, space="PSUM"))

        # ---- load inputs -------------------------------------------------
        cp_sb = singles.tile([PPC, NSEG * 4 * 2], f32)
        nc.sync.dma_start(cp_sb[:], cp_d)
        col_sb = singles.tile([PPC, 4], f32)
        nc.sync.dma_start(col_sb[:], col_d)
        lin_sb = singles.tile([PPC, W], f32)
        nc.sync.dma_start(lin_sb[:], lin_d)
        bas_sb = singles.tile([PPC, 4 * NT], f32)
        nc.sync.dma_start(bas_sb[:], bas_d)
        tri_sb = singles.tile([PPC, PPC], bf16)
        nc.sync.dma_start(tri_sb[:], tri_d)

        # ---- bezier samples: pts[p,s,t,e] = sum_c basis[t,c]*cp[p,s,c,e] -
        cp4 = cp_sb[:].rearrange("p (s c e) -> p s c e", s=NSEG, c=4, e=2)
        bas4 = bas_sb[:].rearrange("p (c t) -> p c t", c=4, t=NT)
        prods = []
        for c in range(4):
            pc = setup.tile([PPC, NSEG, NT, 2], f32, tag=f"prod{c}")
            cpv = cp4[:, :, c, :].unsqueeze(2).broadcast_to([PPC, NSEG, NT, 2])
            bv = (bas4[:, c, :].unsqueeze(1).unsqueeze(3)
                  .broadcast_to([PPC, NSEG, NT, 2]))
            nc.vector.tensor_mul(pc[:], cpv, bv)
            prods.append(pc)
        s01 = setup.tile([PPC, NSEG, NT, 2], f32)
        nc.vector.tensor_add(s01[:], prods[0][:], prods[1][:])
        s23 = setup.tile([PPC, NSEG, NT, 2], f32)
        nc.vector.tensor_add(s23[:], prods[2][:], prods[3][:])
        pts = setup.tile([PPC, NSEG, NT, 2], f32)
        nc.vector.tensor_add(pts[:], s01[:], s23[:])

        ptx = pts[:, :, :, 0]   # [p, 4, 49]
        pty = pts[:, :, :, 1]

        # ---- centers (negated means) ------------------------------------
        sumx = setup.tile([PPC, 1], f32)
        nc.vector.tensor_reduce(sumx[:], ptx, axis=mybir.AxisListType.XY,
                                op=ALU.add)
        sumy = setup.tile([PPC, 1], f32)
        nc.vector.tensor_reduce(sumy[:], pty, axis=mybir.AxisListType.XY,
                                op=ALU.add)
        neg_cx = setup.tile([PPC, 1], f32)
        nc.vector.tensor_scalar_mul(neg_cx[:], sumx[:], -1.0 / NPTS)
        neg_cy = setup.tile([PPC, 1], f32)
        nc.vector.tensor_scalar_mul(neg_cy[:], sumy[:], -1.0 / NPTS)

        # ---- avg radius -> r50 = 50 * mean ||pts - c|| -------------------
        sqx = setup.tile([PPC, NSEG, NT], f32)
        nc.scalar.activation(sqx[:], ptx, ACT.Square, bias=neg_cx[:])
        sqy = setup.tile([PPC, NSEG, NT], f32)
        nc.scalar.activation(sqy[:], pty, ACT.Square, bias=neg_cy[:])
        d2p = setup.tile([PPC, NSEG, NT], f32)
        nc.vector.tensor_add(d2p[:], sqx[:], sqy[:])
        sp = setup.tile([PPC, NSEG, NT], f32)
        rsum = setup.tile([PPC, 1], f32)
        nc.scalar.activation(sp[:], d2p[:], ACT.Sqrt, accum_out=rsum[:])
        r50 = setup.tile([PPC, 1], f32)
        nc.vector.tensor_scalar_mul(r50[:], rsum[:], INV_SOFT / NPTS)

        # ---- per-path alpha and color-diff matmul weights ----------------
        neg_alpha = setup.tile([PPC, 1], f32)
        nc.vector.tensor_scalar_mul(neg_alpha[:], col_sb[:, 3:4], -1.0)

        csh = setup.tile([PPC, 3], f32)       # c_{k-1} (0 for k=0)
        nc.vector.memset(csh[0:1, :], 0.0)
        nc.sync.dma_start(csh[1:PPC, :], col_sb[0:PPC - 1, 0:3])
        d4f = setup.tile([PPC, 4], f32)       # cols 0-2: D, col 3: e_0 (A row)
        nc.vector.tensor_sub(d4f[:, 0:3], csh[:], col_sb[:, 0:3])
        nc.vector.memset(d4f[:, 3:4], 0.0)
        nc.vector.memset(d4f[0:1, 3:4], 1.0)
        d4 = setup.tile([PPC, 4], f32r)
        nc.vector.tensor_copy(d4[:], d4f[:])



        # ---- separable squared distances --------------------------------
        dx2 = singles.tile([PPC, W], f32)
        nc.scalar.activation(dx2[:], lin_sb[:], ACT.Square, bias=neg_cx[:])
        dy2 = singles.tile([PPC, W], f32)
        nc.scalar.activation(dy2[:], lin_sb[:], ACT.Square, bias=neg_cy[:])

        # ---- main loop ---------------------------------------------------
        row0 = 0
        for ch, ch_rows in enumerate(CHUNKS):
            ch_px = ch_rows * W
            t = work.tile([PPC, MAXCH_PX], f32, tag="work")
            for r in range(ch_rows):
                i = row0 + r
                nc.vector.tensor_scalar_add(
                    t[:, r * W:(r + 1) * W], dx2[:], dy2[:, i:i + 1])
            # d2 -> dist -> m0 in place, lg (bf16) for the matmul
            nc.scalar.activation(t[:, :ch_px], t[:, :ch_px], ACT.Sqrt)
            nc.scalar.activation(t[:, :ch_px], t[:, :ch_px], ACT.Sigmoid,
                                 bias=r50[:], scale=-INV_SOFT)
            lg = lgpool.tile([PPC, MAXCH_PX], bf16, tag="lg")
            nc.scalar.activation(lg[:, :ch_px], t[:, :ch_px], ACT.Ln,
                                 bias=1.0, scale=neg_alpha[:])

            ngrp = (ch_px + XBLK - 1) // XBLK
            for g in range(ngrp):
                lo = g * XBLK
                gw = min(XBLK, ch_px - lo)
                px0 = row0 * W + lo

                ss = ps_ss.tile([PPC, XBLK], f32, tag="ss")
                for h in range(0, gw, BLK):
                    hw_ = min(BLK, gw - h)
                    nc.tensor.matmul(ss[:, h:h + hw_], tri_sb[:],
                                     lg[:, lo + h:lo + h + hw_],
                                     start=True, stop=True)
                v = vpool.tile([PPC, XBLK], f32r, tag="v")
                nc.scalar.activation(v[:, :gw], ss[:, :gw], ACT.Exp)

                bp = ps_b.tile([4, XBLK], f32, tag="bp")
                for h in range(0, gw, BLK):
                    hw_ = min(BLK, gw - h)
                    nc.tensor.matmul(bp[:, h:h + hw_], d4[:],
                                     v[:, h:h + hw_],
                                     start=True, stop=True)
                bs = bstage.tile([4, XBLK], f32, tag="bs")
                nc.vector.tensor_copy(bs[:, :gw], bp[:, :gw])
                nc.sync.dma_start(ab_d[:, px0:px0 + gw], bs[:, :gw])
            row0 += ch_rows

    nc.compile()
    return nc


def _get_nc():
    if "nc" not in _compiled:
        _compiled["nc"] = _build_nc()
    return _compiled["nc"]


def _bezier_basis():
    t = np.linspace(0.0, 1.0, NSAMP, dtype=np.float32)[:-1]
    mt = 1.0 - t
    return np.stack([mt ** 3, 3.0 * mt ** 2 * t, 3.0 * mt * t ** 2, t ** 3],
                    axis=-1).astype(np.float32)  # (49, 4)


def kernel(paths_control_points, colors):
    global last_results
    from concourse.bass_utils import run_bass_kernel_spmd

    cp = np.ascontiguousarray(paths_control_points, dtype=np.float32)
    col = np.ascontiguousarray(colors, dtype=np.float32)

    basis = _bezier_basis()                       # (49, 4)
    bas_in = np.broadcast_to(basis.T.reshape(1, 4 * NT),
                             (PPC, 4 * NT)).copy()  # rows: c-major
    lin = np.linspace(0.0, 1.0, W, dtype=np.float32)
    lin_in = np.broadcast_to(lin, (PPC, W)).copy()
    import ml_dtypes
    q = np.arange(PPC)
    tri = (q[:, None] >= q[None, :]).astype(ml_dtypes.bfloat16)  # tri[q,k] = q>=k

    nc = _get_nc()
    in_maps = []
    for s in range(NCORES):
        sl = slice(s * PPC, (s + 1) * PPC)
        in_maps.append({
            "cp": cp[sl].reshape(PPC, NSEG * 4 * 2).copy(),
            "col": col[sl].copy(),
            "lin_t": lin_in,
            "basis": bas_in,
            "tri": tri,
        })

    res = run_bass_kernel_spmd(nc, in_maps, core_ids=list(range(NCORES)))
    last_results = res

    canvas = np.ones((3, H, W), dtype=np.float32)
    for s in range(NCORES):
        ab = res.results[s]["AB"]
        a = ab[3].reshape(H, W)
        b = ab[0:3].reshape(3, H, W)
        c_last = col[s * PPC + PPC - 1, 0:3]
        canvas = canvas * a[None] + b + c_last[:, None, None]
    return canvas.astype(np.float32)


# revision 21
# speedup vs baseline: 1.2970x; 1.0470x over previous
"""Bezier soft-disk renderer on 8 Trainium2 NeuronCores.

Strategy (data-parallel over paths + associative over-compositing):
  Each core gets 128 of the 1024 paths. Front-to-back compositing
    canvas <- canvas*(1-m_p) + c_p*m_p
  is an affine map per pixel, so a shard of 128 consecutive paths
  composes to  canvas_out = canvas_in * A_s + B_s  with
    A_s = prod_p (1-m_p)
    B_s = sum_p c_p m_p prod_{q>p} (1-m_q).
  On-device, per core (paths on the 128 SBUF partitions):
    d2   = (gx-cx)^2 + (gy-cy)^2            (DVE tensor_scalar + ACT Square)
    m0   = Sigmoid(-50*sqrt(d2) + 50*r)     (ACT, per-partition bias)
    lg   = Ln(1 - alpha*m0)                 (ACT, per-partition scale)
    SS_k = sum_{q>=k} lg_q                  (TensorE, triangular ones matmul)
    V    = Exp(SS)                          (ACT)  -> V_0 = A_s
    B_s  = D^T @ V + c_last @ ones          (TensorE; D_k = c_{k-1}-c_k, D_0=-c_0,
                                             4th output row = V_0 = A_s)
  Host work is only shard/gather + the 8-term affine combine.
"""

import sys

if "/opt/trn_rl_repo" not in sys.path:
    sys.path.insert(0, "/opt/trn_rl_repo")

import numpy as np
from contextlib import ExitStack

H = W = 224
NPX = H * W
N_PATHS = 1024
PPC = 128           # paths per core
NCORES = 8
NSEG = 4
NSAMP = 50
NT = NSAMP - 1      # 49 samples per segment
NPTS = NSEG * NT    # 196 samples per path
INV_SOFT = 50.0     # 1/SOFTNESS
CHUNKS = [56, 56, 56, 56]   # rows per chunk (sum = 224)
MAXCH_PX = max(CHUNKS) * W  # biggest chunk, sizes the work tiles
BLK = 512                   # matmul moving-dim block (one PSUM bank)
XBLK = 1024                 # exp/copy/DMA grouping (two PSUM banks)

_compiled = {}
last_results = None


def _build_nc():
    import concourse.tile as tile
    from concourse import bacc, mybir

    f32 = mybir.dt.float32
    f32r = mybir.dt.float32r
    bf16 = mybir.dt.bfloat16
    ACT = mybir.ActivationFunctionType
    ALU = mybir.AluOpType

    nc = bacc.Bacc("TRN2", target_bir_lowering=False, debug=False,
                   num_devices=NCORES)

    cp_d = nc.dram_tensor("cp", [PPC, NSEG * 4 * 2], f32, kind="ExternalInput").ap()
    col_d = nc.dram_tensor("col", [PPC, 4], f32, kind="ExternalInput").ap()
    lin_d = nc.dram_tensor("lin_t", [PPC, W], f32, kind="ExternalInput").ap()
    bas_d = nc.dram_tensor("basis", [PPC, 4 * NT], f32, kind="ExternalInput").ap()
    tri_d = nc.dram_tensor("tri", [PPC, PPC], bf16, kind="ExternalInput").ap()
    ab_d = nc.dram_tensor("AB", [4, NPX], f32, kind="ExternalOutput").ap()

    with ExitStack() as ctx:
        tc = ctx.enter_context(tile.TileContext(nc))

        singles = ctx.enter_context(tc.tile_pool(name="singles", bufs=1))
        setup = ctx.enter_context(tc.tile_pool(name="setup", bufs=1))
        work = ctx.enter_context(tc.tile_pool(name="work", bufs=2))
        lgpool = ctx.enter_context(tc.tile_pool(name="lg", bufs=1))
        vpool = ctx.enter_context(tc.tile_pool(name="vp", bufs=3))
        bstage = ctx.enter_context(tc.tile_pool(name="bst", bufs=3))
        ps_ss = ctx.enter_context(tc.tile_pool(name="pss", bufs=2, space="PSUM"))
        ps_b = ctx.enter_context(tc.tile_pool(name="psb", bufs=2, space="PSUM"))

        # ---- load inputs -------------------------------------------------
        cp_sb = singles.tile([PPC, NSEG * 4 * 2], f32)
        nc.sync.dma_start(cp_sb[:], cp_d)
        col_sb = singles.tile([PPC, 4], f32)
        nc.sync.dma_start(col_sb[:], col_d)
        lin_sb = singles.tile([PPC, W], f32)
        nc.sync.dma_start(lin_sb[:], lin_d)
        bas_sb = singles.tile([PPC, 4 * NT], f32)
        nc.sync.dma_start(bas_sb[:], bas_d)
        tri_sb = singles.tile([PPC, PPC], bf16)
        nc.sync.dma_start(tri_sb[:], tri_d)

        # ---- bezier samples: pts[p,s,t,e] = sum_c basis[t,c]*cp[p,s,c,e] -
        cp4 = cp_sb[:].rearrange("p (s c e) -> p s c e", s=NSEG, c=4, e=2)
        bas4 = bas_sb[:].rearrange("p (c t) -> p c t", c=4, t=NT)
        prods = []
        for c in range(4):
            pc = setup.tile([PPC, NSEG, NT, 2], f32, tag=f"prod{c}")
            cpv = cp4[:, :, c, :].unsqueeze(2).broadcast_to([PPC, NSEG, NT, 2])
            bv = (bas4[:, c, :].unsqueeze(1).unsqueeze(3)
                  .broadcast_to([PPC, NSEG, NT, 2]))
            nc.vector.tensor_mul(pc[:], cpv, bv)
            prods.append(pc)
        s01 = setup.tile([PPC, NSEG, NT, 2], f32)
        nc.vector.tensor_add(s01[:], prods[0][:], prods[1][:])
        s23 = setup.tile([PPC, NSEG, NT, 2], f32)
        nc.vector.tensor_add(s23[:], prods[2][:], prods[3][:])
        pts = setup.tile([PPC, NSEG, NT, 2], f32)
        nc.vector.tensor_add(pts[:], s01[:], s23[:])

        ptx = pts[:, :, :, 0]   # [p, 4, 49]
        pty = pts[:, :, :, 1]

        # ---- centers (negated means) ------------------------------------
        sumx = setup.tile([PPC, 1], f32)
        nc.vector.tensor_reduce(sumx[:], ptx, axis=mybir.AxisListType.XY,
                                op=ALU.add)
        sumy = setup.tile([PPC, 1], f32)
        nc.vector.tensor_reduce(sumy[:], pty, axis=mybir.AxisListType.XY,
                                op=ALU.add)
        neg_cx = setup.tile([PPC, 1], f32)
        nc.vector.tensor_scalar_mul(neg_cx[:], sumx[:], -1.0 / NPTS)
        neg_cy = setup.tile([PPC, 1], f32)
        nc.vector.tensor_scalar_mul(neg_cy[:], sumy[:], -1.0 / NPTS)

        # ---- avg radius -> r50 = 50 * mean ||pts - c|| -------------------
        sqx = setup.tile([PPC, NSEG, NT], f32)
        nc.scalar.activation(sqx[:], ptx, ACT.Square, bias=neg_cx[:])
        sqy = setup.tile([PPC, NSEG, NT], f32)
        nc.scalar.activation(sqy[:], pty, ACT.Square, bias=neg_cy[:])
        d2p = setup.tile([PPC, NSEG, NT], f32)
        nc.vector.tensor_add(d2p[:], sqx[:], sqy[:])
        sp = setup.tile([PPC, NSEG, NT], f32)
        rsum = setup.tile([PPC, 1], f32)
        nc.scalar.activation(sp[:], d2p[:], ACT.Sqrt, accum_out=rsum[:])
        r50 = setup.tile([PPC, 1], f32)
        nc.vector.tensor_scalar_mul(r50[:], rsum[:], INV_SOFT / NPTS)

        # ---- per-path alpha and color-diff matmul weights ----------------
        neg_alpha = setup.tile([PPC, 1], f32)
        nc.vector.tensor_scalar_mul(neg_alpha[:], col_sb[:, 3:4], -1.0)

        csh = setup.tile([PPC, 3], f32)       # c_{k-1} (0 for k=0)
        nc.vector.memset(csh[0:1, :], 0.0)
        nc.sync.dma_start(csh[1:PPC, :], col_sb[0:PPC - 1, 0:3])
        d4f = setup.tile([PPC, 4], f32)       # cols 0-2: D, col 3: e_0 (A row)
        nc.vector.tensor_sub(d4f[:, 0:3], csh[:], col_sb[:, 0:3])
        nc.vector.memset(d4f[:, 3:4], 0.0)
        nc.vector.memset(d4f[0:1, 3:4], 1.0)
        d4 = setup.tile([PPC, 4], f32r)
        nc.vector.tensor_copy(d4[:], d4f[:])



        # ---- separable squared distances --------------------------------
        dx2 = singles.tile([PPC, W], f32)
        nc.scalar.activation(dx2[:], lin_sb[:], ACT.Square, bias=neg_cx[:])
        dy2 = singles.tile([PPC, W], f32)
        nc.scalar.activation(dy2[:], lin_sb[:], ACT.Square, bias=neg_cy[:])

        # ---- main loop ---------------------------------------------------
        row0 = 0
        for ch, ch_rows in enumerate(CHUNKS):
            ch_px = ch_rows * W
            t = work.tile([PPC, MAXCH_PX], f32, tag="work")
            t3 = t[:, :ch_px].rearrange("p (r j) -> p r j", r=ch_rows, j=W)
            dyb = (dy2[:, row0:row0 + ch_rows].unsqueeze(2)
                   .broadcast_to([PPC, ch_rows, W]))
            dxb = dx2[:].unsqueeze(1).broadcast_to([PPC, ch_rows, W])
            nc.vector.tensor_add(t3, dyb, dxb)
            # d2 -> dist -> m0 in place, lg (bf16) for the matmul
            nc.scalar.activation(t[:, :ch_px], t[:, :ch_px], ACT.Sqrt)
            nc.scalar.activation(t[:, :ch_px], t[:, :ch_px], ACT.Sigmoid,
                                 bias=r50[:], scale=-INV_SOFT)
            lg = lgpool.tile([PPC, MAXCH_PX], bf16, tag="lg")
            nc.scalar.activation(lg[:, :ch_px], t[:, :ch_px], ACT.Ln,
                                 bias=1.0, scale=neg_alpha[:])

            ngrp = (ch_px + XBLK - 1) // XBLK
            for g in range(ngrp):
                lo = g * XBLK
                gw = min(XBLK, ch_px - lo)
                px0 = row0 * W + lo

                ss = ps_ss.tile([PPC, XBLK], f32, tag="ss")
                for h in range(0, gw, BLK):
                    hw_ = min(BLK, gw - h)
                    nc.tensor.matmul(ss[:, h:h + hw_], tri_sb[:],
                                     lg[:, lo + h:lo + h + hw_],
                                     start=True, stop=True)
                v = vpool.tile([PPC, XBLK], f32r, tag="v")
                nc.scalar.activation(v[:, :gw], ss[:, :gw], ACT.Exp)

                bp = ps_b.tile([4, XBLK], f32, tag="bp")
                for h in range(0, gw, BLK):
                    hw_ = min(BLK, gw - h)
                    nc.tensor.matmul(bp[:, h:h + hw_], d4[:],
                                     v[:, h:h + hw_],
                                     start=True, stop=True)
                bs = bstage.tile([4, XBLK], f32, tag="bs")
                nc.vector.tensor_copy(bs[:, :gw], bp[:, :gw])
                nc.sync.dma_start(ab_d[:, px0:px0 + gw], bs[:, :gw])
            row0 += ch_rows

    nc.compile()
    return nc


def _get_nc():
    if "nc" not in _compiled:
        _compiled["nc"] = _build_nc()
    return _compiled["nc"]


def _bezier_basis():
    t = np.linspace(0.0, 1.0, NSAMP, dtype=np.float32)[:-1]
    mt = 1.0 - t
    return np.stack([mt ** 3, 3.0 * mt ** 2 * t, 3.0 * mt * t ** 2, t ** 3],
                    axis=-1).astype(np.float32)  # (49, 4)


def kernel(paths_control_points, colors):
    global last_results
    from concourse.bass_utils import run_bass_kernel_spmd

    cp = np.ascontiguousarray(paths_control_points, dtype=np.float32)
    col = np.ascontiguousarray(colors, dtype=np.float32)

    basis = _bezier_basis()                       # (49, 4)
    bas_in = np.broadcast_to(basis.T.reshape(1, 4 * NT),
                             (PPC, 4 * NT)).copy()  # rows: c-major
    lin = np.linspace(0.0, 1.0, W, dtype=np.float32)
    lin_in = np.broadcast_to(lin, (PPC, W)).copy()
    import ml_dtypes
    q = np.arange(PPC)
    tri = (q[:, None] >= q[None, :]).astype(ml_dtypes.bfloat16)  # tri[q,k] = q>=k

    nc = _get_nc()
    in_maps = []
    for s in range(NCORES):
        sl = slice(s * PPC, (s + 1) * PPC)
        in_maps.append({
            "cp": cp[sl].reshape(PPC, NSEG * 4 * 2).copy(),
            "col": col[sl].copy(),
            "lin_t": lin_in,
            "basis": bas_in,
            "tri": tri,
        })

    res = run_bass_kernel_spmd(nc, in_maps, core_ids=list(range(NCORES)))
    last_results = res

    canvas = np.ones((3, H, W), dtype=np.float32)
    for s in range(NCORES):
        ab = res.results[s]["AB"]
        a = ab[3].reshape(H, W)
        b = ab[0:3].reshape(3, H, W)
        c_last = col[s * PPC + PPC - 1, 0:3]
        canvas = canvas * a[None] + b + c_last[:, None, None]
    return canvas.astype(np.float32)
